# revision 21
# baseline (speedup 1.0000x reference)
"""Trainium2 Bass kernel for nn_MultiHeadModel (segment_reduce), 8-core SPMD.

Reference math:
    xp  = x @ Wp + bp                              # [N, 256]
    class_emb[g] = (sum_{i in g} m_i * xp_i) / n_g # [G, 256]  (segment mean)
    h   = concat(repeat(class_emb, C), xp[idx])    # [G*C, 512]
    out = relu(relu(h@W1+b1)@W2+b2) @ W3 + b3      # [G*C, 1]
(edge_attr's projection is dead code - never touched.)

Sharding: data-parallel over graphs, 128 graphs + their nodes + their 2048
output rows per core; weights replicated; no collectives.

Structure (v2):
  *  Constant folding as v1: Wt = Wp@W1_top, Wb = Wp@W1_bot,
     cbias = bp@(W1_top+W1_bot)+b1, so the [N,256] projection never exists.
  *  Segment sum on the PE in fp8e4m3 with perf_mode=DoubleRow: tiles of
     256 masked nodes ([128, 2, d] k-pair layout), one-hot indicator built
     on the HOST and shipped with the stream (no on-device iota/is_equal).
     fp8 x only feeds the segment MEAN (error averages out over ~49 nodes).
  *  x[idx] rows, weights and the whole MLP stay fp16 (fp8 there would not
     average and would blow the error budget).
  *  h1pre stays in PSUM; the per-graph class add is a DVE tensor_tensor
     reading PSUM directly, and cbias rides the Relu activation's bias port
     (kills the 16 PSUM->SBUF copies and 16 bias adds of v1).
  *  Quarters of 32 graphs; each quarter's cls chain + h1 + h2 + head are
     emitted right after its last seg matmul so they fill the PE while the
     rest of the stream lands.
  *  Dual DMA issue queues: constants on the scalar(Activation) HWDGE
     queue, stream + output on sync. One merged output DMA.
  *  A few dependency-free dummy matmuls at kernel start warm the PE HAM
     clock gate (cold PE runs at 1.2 GHz, warm at 2.4) during the DMA ramp.
"""
import numpy as np
import ml_dtypes
from contextlib import ExitStack

import concourse.bacc as bacc
import concourse.mybir as mybir
from concourse.tile import TileContext
from concourse.bass_utils import run_bass_kernel_spmd

M = 8                 # cores
G = 1024              # graphs
C = 16                # classes
GL = G // M           # graphs per core (128)
D = 256
D2 = 512
ROWS = G * C // M     # MLP rows per core (2048)
NQ = 4                # quarters (32 graphs each)
QG = GL // NQ         # graphs per quarter (32)
TN = 256              # nodes per seg tile (DoubleRow: 2x128)
TW = D * 2 + QG * 2   # fp8 bytes/partition per tile: 512 x + 64 ind = 576
SUP = 4               # tiles per stream DMA
NWARM = 8             # PE warm-up dummy matmuls

f32 = mybir.dt.float32
f16 = mybir.dt.float16
f8 = mybir.dt.float8e4
Relu = mybir.ActivationFunctionType.Relu
Copy = mybir.ActivationFunctionType.Copy
DR = mybir.MatmulPerfMode.DoubleRow

# ---- packed constant layout (columns of a [128, CW] fp16 tile) ---------
_off = {}
_c = 0
def _span(name, w):
    global _c
    _off[name] = (_c, w)
    _c += w
# fp16 (2-byte) column units; f32 entries use 2 units/elem
for _k in range(2):
    _span(f"wb{_k}", D2)      # fp16 (Wp@W1_bot) K-chunk      [128, 512]
_span("xgn0", 2 * D2)         # fp16 x[idx]^T cols for chunk 0 [128, 1024]
CP1 = _c                      # end of job-critical prefix
for _k in range(2):
    _span(f"wt{_k}", D2)      # fp16 (Wp@W1_top) K-chunk      [128, 512]
for _k in range(4):
    _span(f"w2{_k}", D)       # fp16                          [128, 256]
for _k in range(2):
    _span(f"w3{_k}", 2)       # fp16 (padded to even width)   [128, 1]
for _k in range(4):
    _span(f"cb{_k}", 2)       # f32                           [128, 1]
for _k in range(2):
    _span(f"b2{_k}", 2)       # f32
_span("b3", 2)                # f32
for _k in range(4):
    _span(f"inv{_k}", 2)      # f32 1/n for graphs [32k, 32k+32) (parts 0..31)
_span("ident", 64)            # f32 eye(32)
CP2 = _c                      # end of cls/mlp constants
for _k in range(1, 4):
    _span(f"xgn{_k}", 2 * D2) # fp16 x[idx]^T cols for chunks 1..3
CW = _c

_cache = {}


def _build(NT, qs, b3f):
    """NT total 256-node tiles; qs = cumulative tile starts per quarter,
    len 5 (qs[0]=0, qs[4]=NT). Quarter q holds graphs [32q, 32q+32)."""
    NS = (NT + SUP - 1) // SUP
    nc = bacc.Bacc(None, target_bir_lowering=False, debug=False)
    xci = nc.dram_tensor("xci", [128, NT * TW], f8, kind="ExternalInput")
    cpk = nc.dram_tensor("cpk", [128, CW], f16, kind="ExternalInput")
    out = nc.dram_tensor("out", [1, ROWS], f32, kind="ExternalOutput")

    with TileContext(nc) as tc, ExitStack() as ctx:
        cst = ctx.enter_context(tc.tile_pool(name="cst", bufs=1))
        stream = ctx.enter_context(tc.tile_pool(name="stream", bufs=NS))
        pseg = ctx.enter_context(tc.tile_pool(name="pseg", bufs=1, space="PSUM"))
        psm = ctx.enter_context(tc.tile_pool(name="psm", bufs=2, space="PSUM"))
        ph1 = ctx.enter_context(tc.tile_pool(name="ph1", bufs=2, space="PSUM"))
        pbig = ctx.enter_context(tc.tile_pool(name="pbig", bufs=3, space="PSUM"))

        # ---- DMA issues ------------------------------------------------
        ctile = cst.tile([128, CW], f16, tag="cpk")
        # scalar HWDGE queue: constants (critical prefix first)
        nc.scalar.dma_start(out=ctile[:, :CP1], in_=cpk[:, :CP1])
        nc.scalar.dma_start(out=ctile[:, CP1:CP2], in_=cpk[:, CP1:CP2])
        # sync queue: the node stream, xgn1..3 interleaved
        stiles = []
        for st in range(NS):
            t0 = st * SUP
            t1 = min(t0 + SUP, NT)
            stile = stream.tile([128, SUP * TW], f8, tag="s")
            nc.sync.dma_start(out=stile[:, :(t1 - t0) * TW],
                              in_=xci[:, t0 * TW:t1 * TW])
            stiles.append(stile)
        for n in range(1, 4):
            o, w = _off[f"xgn{n}"]
            nc.scalar.dma_start(out=ctile[:, o:o + w], in_=cpk[:, o:o + w])

        def cs(name, dt=f16):
            o, w = _off[name]
            ap = ctile[:, o:o + w]
            return ap.bitcast(dt) if dt is not f16 else ap

        # ---- PE warm-up (no data deps) ---------------------------------
        dum = cst.tile([128, 256], f16, tag="dum")
        nc.vector.memset(dum[:], 0.0)
        for _ in range(NWARM):
            pd = pbig.tile([128, 256], f32, tag="big", padded_shape=[128, 512])
            nc.tensor.matmul(out=pd[:], lhsT=dum[:, :128], rhs=dum[:, :256],
                             start=True, stop=True)

        out_sb = cst.tile([1, ROWS], f32, tag="osb")

        def quarter_done(q, psq):
            # class mean -> transpose -> cls1 = mean@Wt (cb rides h1's bias)
            sxs = cst.tile([32, D], f32, tag="sxs", bufs=2)
            nc.vector.tensor_scalar_mul(out=sxs[:], in0=psq[:],
                                        scalar1=cs(f"inv{q}", f32)[:32, :1])
            sxT = []
            for c2 in range(2):
                pt = psm.tile([128, 32], f32, tag="sm")
                nc.tensor.transpose(out=pt[:], in_=sxs[:, c2 * 128:(c2 + 1) * 128],
                                    identity=cs("ident", f32)[:32, :])
                stq = cst.tile([128, 32], f16, tag=f"sxT{c2}")
                nc.vector.tensor_copy(out=stq[:], in_=pt[:])
                sxT.append(stq)
            cls1b = []
            for m1 in range(4):
                p1 = psm.tile([128, 32], f32, tag="sm")
                for k2 in range(2):
                    nc.tensor.matmul(out=p1[:],
                                     lhsT=cs(f"wt{k2}")[:, m1 * 128:(m1 + 1) * 128],
                                     rhs=sxT[k2][:], start=(k2 == 0), stop=(k2 == 1))
                cb = cst.tile([128, 32], f32, tag="cls1b", bufs=4)
                nc.vector.tensor_copy(out=cb[:], in_=p1[:])
                cls1b.append(cb)
            # h1 = relu(Wb.T@x[idx] + cls1 + cbias), kept in PSUM until relu
            h1t = []
            for m1 in range(4):
                h = h1p_sb.pop((q, m1))
                nc.vector.tensor_tensor(
                    out=h[:],
                    in0=h[:],
                    in1=cls1b[m1][:, :, None].to_broadcast([128, QG, C]),
                    op=mybir.AluOpType.add,
                )
                nc.gpsimd.tensor_scalar(out=h[:], in0=h[:],
                                        scalar1=cs(f"cb{m1}", f32)[:, :1],
                                        scalar2=0.0,
                                        op0=mybir.AluOpType.add,
                                        op1=mybir.AluOpType.max)
                h1t.append(h)
            # h2 = relu(h1@W2 + b2), k4-outer
            p2t = [pbig.tile([128, 512], f32, tag="big", name=f"p2_{m2}")
                   for m2 in range(2)]
            for k4 in range(4):
                for m2 in range(2):
                    nc.tensor.matmul(out=p2t[m2][:],
                                     lhsT=cs(f"w2{k4}")[:, m2 * 128:(m2 + 1) * 128],
                                     rhs=h1t[k4][:], start=(k4 == 0), stop=(k4 == 3))
            h2t = []
            for m2 in range(2):
                h = cst.tile([128, 512], f16, tag="h2", bufs=3)
                nc.scalar.activation(out=h[:], in_=p2t[m2][:], func=Relu,
                                     bias=cs(f"b2{m2}", f32)[:, :1])
                h2t.append(h)
            # out = h2@W3 + b3
            po = pbig.tile([1, 512], f32, tag="big")
            for k2 in range(2):
                nc.tensor.matmul(out=po[:], lhsT=cs(f"w3{k2}")[:, :1],
                                 rhs=h2t[k2][:], start=(k2 == 0), stop=(k2 == 1))
            nc.vector.tensor_scalar_add(out=out_sb[:1, q * 512:(q + 1) * 512],
                                        in0=po[:], scalar1=b3f)
            nc.sync.dma_start(out=out[:1, q * 512:(q + 1) * 512],
                              in_=out_sb[:1, q * 512:(q + 1) * 512])

        # ---- stream: seg matmuls, h1pre jobs paced alongside ------------
        h1p_sb = {}
        jobs = [(q_, m1_) for q_ in range(4) for m1_ in range(4)]
        jobs_done = 0

        def emit_job():
            nonlocal jobs_done
            q_, m1_ = jobs[jobs_done]
            jobs_done += 1
            xo_, _ = _off[f"xgn{q_}"]
            hp = ph1.tile([128, 512], f32, tag="h1p", name=f"hp{q_}{m1_}")
            for k2 in range(2):
                nc.tensor.matmul(out=hp[:],
                                 lhsT=cs(f"wb{k2}")[:, m1_ * 128:(m1_ + 1) * 128],
                                 rhs=ctile[:, xo_ + k2 * D2:xo_ + (k2 + 1) * D2],
                                 start=(k2 == 0), stop=(k2 == 1))
            t = cst.tile([128, 512], f16, tag="h1p_sb", bufs=16,
                         name=f"h1sb{q_}{m1_}")
            nc.scalar.activation(out=t[:], in_=hp[:], func=Copy)
            h1p_sb[(q_, m1_)] = t

        psq = None
        for ti in range(NT):
            q = sum(ti >= qs[j] for j in range(1, 4))
            if ti == qs[q]:
                psq = pseg.tile([QG, D], f32, tag="psQ")
            stile = stiles[ti // SUP]
            lo = (ti % SUP) * TW
            xap = stile[:, lo:lo + 2 * D].rearrange("p (i d) -> p i d", i=2)
            iap = stile[:, lo + 2 * D:lo + TW].rearrange("p (i g) -> p i g", i=2)
            nc.tensor.matmul(out=psq[:], lhsT=iap, rhs=xap,
                             start=(ti == qs[q]), stop=(ti == qs[q + 1] - 1),
                             perf_mode=DR)
            if ti >= 3:
                while jobs_done < min(16, ((ti + 1) * 16) // NT + 2):
                    emit_job()
            if ti == qs[q + 1] - 1:
                while jobs_done < 4 * (q + 1):
                    emit_job()
                quarter_done(q, psq)

    nc.compile()
    return nc


def _pack_consts(Wt, Wb, W2, W3, cbias, b2, b3, invq, xgt):
    cpk = np.zeros((128, CW), np.float16)
    def put16(name, arr):
        o, w = _off[name]
        a = np.ascontiguousarray(arr, dtype=np.float16)
        cpk[:a.shape[0], o:o + a.shape[1]] = a
    def put32(name, arr):
        o, w = _off[name]
        a = np.ascontiguousarray(arr, dtype=np.float32).view(np.float16)
        cpk[:a.shape[0], o:o + a.shape[1]] = a
    def put8(name, arr):
        o, w = _off[name]
        a = np.ascontiguousarray(arr.astype(ml_dtypes.float8_e4m3)).view(np.uint8)
        a = np.ascontiguousarray(a).view(np.float16)
        cpk[:a.shape[0], o:o + a.shape[1]] = a
    for k in range(2):
        put16(f"wt{k}", Wt[k * 128:(k + 1) * 128])
        put16(f"wb{k}", Wb[k * 128:(k + 1) * 128])
        put16(f"w3{k}", W3[k * 128:(k + 1) * 128])
        put32(f"b2{k}", b2[k * 128:(k + 1) * 128, None])
    for k in range(4):
        put32(f"cb{k}", cbias[k * 128:(k + 1) * 128, None])
    for k in range(4):
        put16(f"w2{k}", W2[k * 128:(k + 1) * 128])
        # x[idx]^T columns for chunk k: k-chunk0 block | k-chunk1 block
        put16(f"xgn{k}", np.concatenate(
            [xgt[0:128, k * D2:(k + 1) * D2],
             xgt[128:256, k * D2:(k + 1) * D2]], axis=1))
    put32("b3", b3[None, :1])
    for q in range(4):
        put32(f"inv{q}", invq[q][:, None])
    put32("ident", np.eye(32, dtype=np.float32))
    return np.ascontiguousarray(cpk)


def kernel(x, edge_attr, batch, target_node_mask, true_nodes_idx,
           Wp, bp, W1, b1, W2, b2, W3, b3,
           num_graphs=G, num_classes=C, **_):
    x = np.ascontiguousarray(np.asarray(x), dtype=np.float32)
    batch = np.asarray(batch).astype(np.int64)
    mask = np.asarray(target_node_mask).astype(bool)
    idx = np.asarray(true_nodes_idx).astype(np.int64)
    Wp = np.asarray(Wp, np.float32)
    W1 = np.asarray(W1, np.float32)
    W2 = np.ascontiguousarray(np.asarray(W2), np.float32)
    W3 = np.ascontiguousarray(np.asarray(W3), np.float32)
    bp = np.asarray(bp, np.float32)
    b1 = np.asarray(b1, np.float32)
    b2 = np.asarray(b2, np.float32)
    b3 = np.asarray(b3, np.float32)

    # constant-fold the initial projection into W1's two halves
    Wt = (Wp @ W1[:D]).astype(np.float32)          # [256, 512]
    Wb = (Wp @ W1[D:]).astype(np.float32)          # [256, 512]
    cbias = (bp @ (W1[:D] + W1[D:]) + b1).astype(np.float32)  # [512]

    ncount = np.bincount(batch[mask], minlength=G).astype(np.float32)
    with np.errstate(divide="ignore"):
        inv_all = (np.float32(1.0) / ncount).astype(np.float32)

    core = batch // GL
    quarter = (batch % GL) // QG
    selq = [[np.flatnonzero((core == k) & mask & (quarter == q))
             for q in range(4)] for k in range(M)]
    BQ = [max(1, max((len(selq[k][q]) + TN - 1) // TN for k in range(M)))
          for q in range(4)]
    qs = [0, BQ[0], BQ[0] + BQ[1], BQ[0] + BQ[1] + BQ[2], sum(BQ)]
    NT = qs[4]

    key = (NT, tuple(qs), float(b3[0]))
    if key not in _cache:
        _cache[key] = _build(NT, qs, float(b3[0]))
    nc = _cache[key]

    xf8 = x.astype(ml_dtypes.float8_e4m3)
    in_maps = []
    for k in range(M):
        buf = np.zeros((NT, 128, TW), ml_dtypes.float8_e4m3)
        for q in range(4):
            rows = selq[k][q]
            n = len(rows)
            nn = BQ[q] * TN
            xb = np.zeros((nn, D), ml_dtypes.float8_e4m3)
            xb[:n] = xf8[rows]
            ib = np.zeros((nn, QG), ml_dtypes.float8_e4m3)
            ib[np.arange(n), batch[rows] - k * GL - q * QG] = 1.0
            # node j of tile t -> partition j%128, k-pair half j//128
            buf[qs[q]:qs[q] + BQ[q], :, :2 * D] = (
                xb.reshape(BQ[q], 2, 128, D).transpose(0, 2, 1, 3)
                  .reshape(BQ[q], 128, 2 * D))
            buf[qs[q]:qs[q] + BQ[q], :, 2 * D:] = (
                ib.reshape(BQ[q], 2, 128, QG).transpose(0, 2, 1, 3)
                  .reshape(BQ[q], 128, 2 * QG))
        xci = np.ascontiguousarray(
            buf.transpose(1, 0, 2).reshape(128, NT * TW))
        invq = [inv_all[k * GL + QG * q:k * GL + QG * (q + 1)] for q in range(4)]
        xgt = np.ascontiguousarray(x[idx[k * ROWS:(k + 1) * ROWS]].T)
        cpk = _pack_consts(Wt, Wb, W2, W3, cbias, b2, b3, invq, xgt)
        in_maps.append(dict(xci=xci, cpk=cpk))

    res = run_bass_kernel_spmd(nc, in_maps, list(range(M)))
    out = np.concatenate([res.results[k]["out"].reshape(ROWS) for k in range(M)])
    return out.reshape(G * C, 1).astype(np.float32)


# revision 23
# speedup vs baseline: 3.2009x; 3.2009x over previous
"""Trainium2 Bass kernel for nn_MultiHeadModel (segment_reduce), 8-core SPMD.

Reference math:
    xp  = x @ Wp + bp                              # [N, 256]
    class_emb[g] = (sum_{i in g} m_i * xp_i) / n_g # [G, 256]  (segment mean)
    h   = concat(repeat(class_emb, C), xp[idx])    # [G*C, 512]
    out = relu(relu(h@W1+b1)@W2+b2) @ W3 + b3      # [G*C, 1]
(edge_attr's projection is dead code - never touched.)

Sharding: data-parallel over graphs, 128 graphs + their nodes + their 2048
output rows per core; weights replicated; no collectives.

Structure (v2):
  *  Constant folding as v1: Wt = Wp@W1_top, Wb = Wp@W1_bot,
     cbias = bp@(W1_top+W1_bot)+b1, so the [N,256] projection never exists.
  *  Segment sum on the PE in fp8e4m3 with perf_mode=DoubleRow: tiles of
     256 masked nodes ([128, 2, d] k-pair layout), one-hot indicator built
     on the HOST and shipped with the stream (no on-device iota/is_equal).
     fp8 x only feeds the segment MEAN (error averages out over ~49 nodes).
  *  x[idx] rows, weights and the whole MLP stay fp16 (fp8 there would not
     average and would blow the error budget).
  *  h1pre stays in PSUM; the per-graph class add is a DVE tensor_tensor
     reading PSUM directly, and cbias rides the Relu activation's bias port
     (kills the 16 PSUM->SBUF copies and 16 bias adds of v1).
  *  Quarters of 32 graphs; each quarter's cls chain + h1 + h2 + head are
     emitted right after its last seg matmul so they fill the PE while the
     rest of the stream lands.
  *  Dual DMA issue queues: constants on the scalar(Activation) HWDGE
     queue, stream + output on sync. One merged output DMA.
  *  A few dependency-free dummy matmuls at kernel start warm the PE HAM
     clock gate (cold PE runs at 1.2 GHz, warm at 2.4) during the DMA ramp.
"""
import numpy as np
import ml_dtypes
from contextlib import ExitStack

import concourse.bacc as bacc
import concourse.mybir as mybir
from concourse.tile import TileContext
from concourse.bass_utils import run_bass_kernel_spmd

M = 8                 # cores
G = 1024              # graphs
C = 16                # classes
GL = G // M           # graphs per core (128)
D = 256
D2 = 512
ROWS = G * C // M     # MLP rows per core (2048)
NQ = 4                # quarters (32 graphs each)
QG = GL // NQ         # graphs per quarter (32)
TN = 256              # nodes per seg tile (DoubleRow: 2x128)
TW = D * 2 + QG * 2   # fp8 bytes/partition per tile: 512 x + 64 ind = 576
SUP = 4               # tiles per stream DMA
NWARM = 8             # PE warm-up dummy matmuls

f32 = mybir.dt.float32
f16 = mybir.dt.float16
f8 = mybir.dt.float8e4
Relu = mybir.ActivationFunctionType.Relu
Copy = mybir.ActivationFunctionType.Copy
DR = mybir.MatmulPerfMode.DoubleRow

# ---- packed constant layout (columns of a [128, CW] fp16 tile) ---------
_off = {}
_c = 0
def _span(name, w):
    global _c
    _off[name] = (_c, w)
    _c += w
# fp16 (2-byte) column units; f32 entries use 2 units/elem
for _k in range(2):
    _span(f"wb{_k}", D2)      # fp16 (Wp@W1_bot) K-chunk      [128, 512]
_span("xgn0", 2 * D2)         # fp16 x[idx]^T cols for chunk 0 [128, 1024]
CP1 = _c                      # end of job-critical prefix
for _k in range(2):
    _span(f"wt{_k}", D2)      # fp16 (Wp@W1_top) K-chunk      [128, 512]
for _k in range(4):
    _span(f"w2{_k}", D)       # fp16                          [128, 256]
for _k in range(2):
    _span(f"w3{_k}", 2)       # fp16 (padded to even width)   [128, 1]
for _k in range(4):
    _span(f"cb{_k}", 2)       # f32                           [128, 1]
for _k in range(2):
    _span(f"b2{_k}", 2)       # f32
_span("b3", 2)                # f32
for _k in range(4):
    _span(f"inv{_k}", 2)      # f32 1/n for graphs [32k, 32k+32) (parts 0..31)
_span("ident", 64)            # f32 eye(32)
CP2 = _c                      # end of cls/mlp constants
for _k in range(1, 4):
    _span(f"xgn{_k}", 2 * D2) # fp16 x[idx]^T cols for chunks 1..3
CW = _c

_cache = {}


def _build(NT, qs, b3f):
    """NT total 256-node tiles; qs = cumulative tile starts per quarter,
    len 5 (qs[0]=0, qs[4]=NT). Quarter q holds graphs [32q, 32q+32)."""
    NS = (NT + SUP - 1) // SUP
    nc = bacc.Bacc(None, target_bir_lowering=False, debug=False)
    xci = nc.dram_tensor("xci", [128, NT * TW], f8, kind="ExternalInput")
    cpk = nc.dram_tensor("cpk", [128, CW], f16, kind="ExternalInput")
    out = nc.dram_tensor("out", [1, ROWS], f32, kind="ExternalOutput")

    with TileContext(nc) as tc, ExitStack() as ctx:
        cst = ctx.enter_context(tc.tile_pool(name="cst", bufs=1))
        stream = ctx.enter_context(tc.tile_pool(name="stream", bufs=NS))
        pseg = ctx.enter_context(tc.tile_pool(name="pseg", bufs=1, space="PSUM"))
        psm = ctx.enter_context(tc.tile_pool(name="psm", bufs=2, space="PSUM"))
        ph1 = ctx.enter_context(tc.tile_pool(name="ph1", bufs=2, space="PSUM"))
        pbig = ctx.enter_context(tc.tile_pool(name="pbig", bufs=3, space="PSUM"))

        # ---- DMA issues ------------------------------------------------
        ctile = cst.tile([128, CW], f16, tag="cpk")
        # scalar HWDGE queue: constants (critical prefix first)
        nc.scalar.dma_start(out=ctile[:, :CP1], in_=cpk[:, :CP1])
        nc.scalar.dma_start(out=ctile[:, CP1:CP2], in_=cpk[:, CP1:CP2])
        # sync queue: the node stream, xgn1..3 interleaved
        stiles = []
        for st in range(NS):
            t0 = st * SUP
            t1 = min(t0 + SUP, NT)
            stile = stream.tile([128, SUP * TW], f8, tag="s")
            nc.sync.dma_start(out=stile[:, :(t1 - t0) * TW],
                              in_=xci[:, t0 * TW:t1 * TW])
            stiles.append(stile)
        for n in range(1, 4):
            o, w = _off[f"xgn{n}"]
            nc.scalar.dma_start(out=ctile[:, o:o + w], in_=cpk[:, o:o + w])

        def cs(name, dt=f16):
            o, w = _off[name]
            ap = ctile[:, o:o + w]
            return ap.bitcast(dt) if dt is not f16 else ap

        # ---- PE warm-up (no data deps) ---------------------------------
        dum = cst.tile([128, 256], f16, tag="dum")
        nc.vector.memset(dum[:], 0.0)
        for _ in range(NWARM):
            pd = pbig.tile([128, 256], f32, tag="big", padded_shape=[128, 512])
            nc.tensor.matmul(out=pd[:], lhsT=dum[:, :128], rhs=dum[:, :256],
                             start=True, stop=True)

        out_sb = cst.tile([1, ROWS], f32, tag="osb")

        def quarter_done(q, psq):
            # class mean -> transpose -> cls1 = mean@Wt (cb rides h1's bias)
            sxs = cst.tile([32, D], f32, tag="sxs", bufs=2)
            nc.vector.tensor_scalar_mul(out=sxs[:], in0=psq[:],
                                        scalar1=cs(f"inv{q}", f32)[:32, :1])
            sxT = []
            for c2 in range(2):
                pt = psm.tile([128, 32], f32, tag="sm")
                nc.tensor.transpose(out=pt[:], in_=sxs[:, c2 * 128:(c2 + 1) * 128],
                                    identity=cs("ident", f32)[:32, :])
                stq = cst.tile([128, 32], f16, tag=f"sxT{c2}")
                nc.vector.tensor_copy(out=stq[:], in_=pt[:])
                sxT.append(stq)
            cls1b = []
            for m1 in range(4):
                p1 = psm.tile([128, 32], f32, tag="sm")
                for k2 in range(2):
                    nc.tensor.matmul(out=p1[:],
                                     lhsT=cs(f"wt{k2}")[:, m1 * 128:(m1 + 1) * 128],
                                     rhs=sxT[k2][:], start=(k2 == 0), stop=(k2 == 1))
                cb = cst.tile([128, 32], f32, tag="cls1b", bufs=4)
                nc.vector.tensor_copy(out=cb[:], in_=p1[:])
                cls1b.append(cb)
            # h1 = relu(Wb.T@x[idx] + cls1 + cbias), kept in PSUM until relu
            h1t = []
            for m1 in range(4):
                h = h1p_sb.pop((q, m1))
                nc.vector.tensor_tensor(
                    out=h[:],
                    in0=h[:],
                    in1=cls1b[m1][:, :, None].to_broadcast([128, QG, C]),
                    op=mybir.AluOpType.add,
                )
                nc.scalar.activation(out=h[:], in_=h[:], func=Relu)
                h1t.append(h)
            # h2 = relu(h1@W2 + b2), k4-outer
            p2t = [pbig.tile([128, 512], f32, tag="big", name=f"p2_{m2}")
                   for m2 in range(2)]
            for k4 in range(4):
                for m2 in range(2):
                    nc.tensor.matmul(out=p2t[m2][:],
                                     lhsT=cs(f"w2{k4}")[:, m2 * 128:(m2 + 1) * 128],
                                     rhs=h1t[k4][:], start=(k4 == 0), stop=(k4 == 3))
            h2t = []
            for m2 in range(2):
                h = cst.tile([128, 512], f16, tag="h2", bufs=3)
                if m2 == 0:
                    nc.scalar.activation(out=h[:], in_=p2t[m2][:], func=Relu,
                                         bias=cs(f"b2{m2}", f32)[:, :1])
                else:
                    nc.vector.tensor_scalar(out=h[:], in0=p2t[m2][:],
                                            scalar1=cs(f"b2{m2}", f32)[:, :1],
                                            scalar2=0.0,
                                            op0=mybir.AluOpType.add,
                                            op1=mybir.AluOpType.max)
                h2t.append(h)
            # out = h2@W3 + b3
            po = pbig.tile([1, 512], f32, tag="big")
            for k2 in range(2):
                nc.tensor.matmul(out=po[:], lhsT=cs(f"w3{k2}")[:, :1],
                                 rhs=h2t[k2][:], start=(k2 == 0), stop=(k2 == 1))
            nc.vector.tensor_scalar_add(out=out_sb[:1, q * 512:(q + 1) * 512],
                                        in0=po[:], scalar1=b3f)
            nc.sync.dma_start(out=out[:1, q * 512:(q + 1) * 512],
                              in_=out_sb[:1, q * 512:(q + 1) * 512])

        # ---- stream: seg matmuls, h1pre jobs paced alongside ------------
        h1p_sb = {}
        jobs = [(q_, m1_) for q_ in range(4) for m1_ in range(4)]
        jobs_done = 0

        def emit_job():
            nonlocal jobs_done
            q_, m1_ = jobs[jobs_done]
            jobs_done += 1
            xo_, _ = _off[f"xgn{q_}"]
            hp = ph1.tile([128, 512], f32, tag="h1p", name=f"hp{q_}{m1_}")
            for k2 in range(2):
                nc.tensor.matmul(out=hp[:],
                                 lhsT=cs(f"wb{k2}")[:, m1_ * 128:(m1_ + 1) * 128],
                                 rhs=ctile[:, xo_ + k2 * D2:xo_ + (k2 + 1) * D2],
                                 start=(k2 == 0), stop=(k2 == 1))
            t = cst.tile([128, 512], f16, tag="h1p_sb", bufs=16,
                         name=f"h1sb{q_}{m1_}")
            nc.scalar.activation(out=t[:], in_=hp[:], func=mybir.ActivationFunctionType.Identity,
                                 bias=cs(f"cb{m1_}", f32)[:, :1])
            h1p_sb[(q_, m1_)] = t

        psq = None
        for ti in range(NT):
            q = sum(ti >= qs[j] for j in range(1, 4))
            if ti == qs[q]:
                psq = pseg.tile([QG, D], f32, tag="psQ")
            stile = stiles[ti // SUP]
            lo = (ti % SUP) * TW
            xap = stile[:, lo:lo + 2 * D].rearrange("p (i d) -> p i d", i=2)
            iap = stile[:, lo + 2 * D:lo + TW].rearrange("p (i g) -> p i g", i=2)
            nc.tensor.matmul(out=psq[:], lhsT=iap, rhs=xap,
                             start=(ti == qs[q]), stop=(ti == qs[q + 1] - 1),
                             perf_mode=DR)
            if ti >= 3:
                while jobs_done < min(16, ((ti + 1) * 16) // NT + 2):
                    emit_job()
            if ti == qs[q + 1] - 1:
                while jobs_done < 4 * (q + 1):
                    emit_job()
                quarter_done(q, psq)

    nc.compile()
    return nc


def _pack_consts(Wt, Wb, W2, W3, cbias, b2, b3, invq, xgt):
    cpk = np.zeros((128, CW), np.float16)
    def put16(name, arr):
        o, w = _off[name]
        a = np.ascontiguousarray(arr, dtype=np.float16)
        cpk[:a.shape[0], o:o + a.shape[1]] = a
    def put32(name, arr):
        o, w = _off[name]
        a = np.ascontiguousarray(arr, dtype=np.float32).view(np.float16)
        cpk[:a.shape[0], o:o + a.shape[1]] = a
    def put8(name, arr):
        o, w = _off[name]
        a = np.ascontiguousarray(arr.astype(ml_dtypes.float8_e4m3)).view(np.uint8)
        a = np.ascontiguousarray(a).view(np.float16)
        cpk[:a.shape[0], o:o + a.shape[1]] = a
    for k in range(2):
        put16(f"wt{k}", Wt[k * 128:(k + 1) * 128])
        put16(f"wb{k}", Wb[k * 128:(k + 1) * 128])
        put16(f"w3{k}", W3[k * 128:(k + 1) * 128])
        put32(f"b2{k}", b2[k * 128:(k + 1) * 128, None])
    for k in range(4):
        put32(f"cb{k}", cbias[k * 128:(k + 1) * 128, None])
    for k in range(4):
        put16(f"w2{k}", W2[k * 128:(k + 1) * 128])
        # x[idx]^T columns for chunk k: k-chunk0 block | k-chunk1 block
        put16(f"xgn{k}", np.concatenate(
            [xgt[0:128, k * D2:(k + 1) * D2],
             xgt[128:256, k * D2:(k + 1) * D2]], axis=1))
    put32("b3", b3[None, :1])
    for q in range(4):
        put32(f"inv{q}", invq[q][:, None])
    put32("ident", np.eye(32, dtype=np.float32))
    return np.ascontiguousarray(cpk)


def kernel(x, edge_attr, batch, target_node_mask, true_nodes_idx,
           Wp, bp, W1, b1, W2, b2, W3, b3,
           num_graphs=G, num_classes=C, **_):
    x = np.ascontiguousarray(np.asarray(x), dtype=np.float32)
    batch = np.asarray(batch).astype(np.int64)
    mask = np.asarray(target_node_mask).astype(bool)
    idx = np.asarray(true_nodes_idx).astype(np.int64)
    Wp = np.asarray(Wp, np.float32)
    W1 = np.asarray(W1, np.float32)
    W2 = np.ascontiguousarray(np.asarray(W2), np.float32)
    W3 = np.ascontiguousarray(np.asarray(W3), np.float32)
    bp = np.asarray(bp, np.float32)
    b1 = np.asarray(b1, np.float32)
    b2 = np.asarray(b2, np.float32)
    b3 = np.asarray(b3, np.float32)

    # constant-fold the initial projection into W1's two halves
    Wt = (Wp @ W1[:D]).astype(np.float32)          # [256, 512]
    Wb = (Wp @ W1[D:]).astype(np.float32)          # [256, 512]
    cbias = (bp @ (W1[:D] + W1[D:]) + b1).astype(np.float32)  # [512]

    ncount = np.bincount(batch[mask], minlength=G).astype(np.float32)
    with np.errstate(divide="ignore"):
        inv_all = (np.float32(1.0) / ncount).astype(np.float32)

    core = batch // GL
    quarter = (batch % GL) // QG
    selq = [[np.flatnonzero((core == k) & mask & (quarter == q))
             for q in range(4)] for k in range(M)]
    BQ = [max(1, max((len(selq[k][q]) + TN - 1) // TN for k in range(M)))
          for q in range(4)]
    qs = [0, BQ[0], BQ[0] + BQ[1], BQ[0] + BQ[1] + BQ[2], sum(BQ)]
    NT = qs[4]

    key = (NT, tuple(qs), float(b3[0]))
    if key not in _cache:
        _cache[key] = _build(NT, qs, float(b3[0]))
    nc = _cache[key]

    xf8 = x.astype(ml_dtypes.float8_e4m3)
    in_maps = []
    for k in range(M):
        buf = np.zeros((NT, 128, TW), ml_dtypes.float8_e4m3)
        for q in range(4):
            rows = selq[k][q]
            n = len(rows)
            nn = BQ[q] * TN
            xb = np.zeros((nn, D), ml_dtypes.float8_e4m3)
            xb[:n] = xf8[rows]
            ib = np.zeros((nn, QG), ml_dtypes.float8_e4m3)
            ib[np.arange(n), batch[rows] - k * GL - q * QG] = 1.0
            # node j of tile t -> partition j%128, k-pair half j//128
            buf[qs[q]:qs[q] + BQ[q], :, :2 * D] = (
                xb.reshape(BQ[q], 2, 128, D).transpose(0, 2, 1, 3)
                  .reshape(BQ[q], 128, 2 * D))
            buf[qs[q]:qs[q] + BQ[q], :, 2 * D:] = (
                ib.reshape(BQ[q], 2, 128, QG).transpose(0, 2, 1, 3)
                  .reshape(BQ[q], 128, 2 * QG))
        xci = np.ascontiguousarray(
            buf.transpose(1, 0, 2).reshape(128, NT * TW))
        invq = [inv_all[k * GL + QG * q:k * GL + QG * (q + 1)] for q in range(4)]
        xgt = np.ascontiguousarray(x[idx[k * ROWS:(k + 1) * ROWS]].T)
        cpk = _pack_consts(Wt, Wb, W2, W3, cbias, b2, b3, invq, xgt)
        in_maps.append(dict(xci=xci, cpk=cpk))

    res = run_bass_kernel_spmd(nc, in_maps, list(range(M)))
    out = np.concatenate([res.results[k]["out"].reshape(ROWS) for k in range(M)])
    return out.reshape(G * C, 1).astype(np.float32)
